# revision 2
# baseline (speedup 1.0000x reference)
"""GroupedPNMLP forward on 8 Trainium2 NeuronCores (pure data parallel).

Per-node 2-layer MLP (32->32->1), 24 nodes in 6 groups of 4, with a
group-validity mask and node permutation.  Full inputs in, full output out;
samples are sharded N/8 per core, tiny weights replicated.

Per-core pipeline (chunks of C=2048 samples):
  DMA h in two half-chunks (natural [s, n*c] layout)
  -> ReLU in place (split GpSimd/ACT)
  -> DVE 32x32 block-transpose (channels onto partitions)
  -> mm1: 16 concurrent 32x32 TensorE sub-tile matmuls (tile_position);
     per-node weights replicated across the 4 row blocks; 6 waves of 4 nodes;
     each (row i, col jj) tile writes its own full PSUM bank (N=512) --
     hardware forbids two matmul writers on one (partition-range, bank).
  -> ACT: fused ReLU+b1 PSUM->SBUF copy
  -> mm2: block-structured W2 [128x32], col-position 32*i per sample block,
     accumulated over the 6 waves into one PSUM bank [128, 512]
  -> +b2, x group-valid mask (strided-AP reduces over the arithmetic-sequence
     group columns), DVE block-transpose back
  -> DMA out (natural layout)
"""

import numpy as np

import concourse.bass as bass
from concourse import bacc
import concourse.tile as tile
from concourse import mybir
from concourse.bass_utils import run_bass_kernel_spmd

F32 = mybir.dt.float32
I32 = mybir.dt.int32

GROUPING = np.array(
    [[0, 3, 6, 9], [1, 4, 7, 10], [2, 5, 8, 11],
     [12, 13, 14, 15], [16, 18, 20, 22], [17, 19, 21, 23]], dtype=np.int32)

N_CORES = 8
S_TOT = 131072
S = S_TOT // N_CORES      # 16384 samples per core
NODES = 24
CH = 32                   # in channels = hidden dim
C = 2048                  # samples per chunk
NSUB = C // 128           # 16 sub-tiles of 128 samples
HSUB = NSUB // 2          # 8 sub-tiles per half-chunk
CQ = C // 4               # 512 samples per i-block = one matmul N = one bank
NCHUNK = S // C           # 8
NW = 6                    # waves of 4 nodes


def _build_program():
    nc = bacc.Bacc(None, target_bir_lowering=False)

    h = nc.dram_tensor("h", [S, NODES * CH], F32, kind="ExternalInput")
    valid = nc.dram_tensor("valid", [S, NODES], I32, kind="ExternalInput")
    w1rep = nc.dram_tensor("w1rep", [128, NW, 4, CH], F32, kind="ExternalInput")
    w2blk = nc.dram_tensor("w2blk", [128, NW, 32], F32, kind="ExternalInput")
    b1col = nc.dram_tensor("b1col", [128, NW], F32, kind="ExternalInput")
    b2col = nc.dram_tensor("b2col", [128, 1], F32, kind="ExternalInput")
    out = nc.dram_tensor("out", [S, NODES], F32, kind="ExternalOutput")

    with tile.TileContext(nc) as tc:
        with (
            tc.tile_pool(name="singles", bufs=1) as singles,
            tc.tile_pool(name="xp", bufs=2) as xp,
            tc.tile_pool(name="xtp", bufs=2) as xtp,
            tc.tile_pool(name="hidp", bufs=2) as hidp,
            tc.tile_pool(name="vp", bufs=2) as vp,
            tc.tile_pool(name="op", bufs=2) as op,
            tc.tile_pool(name="pha_pool", bufs=2, space="PSUM") as pha_pool,
            tc.tile_pool(name="phb_pool", bufs=1, space="PSUM") as phb_pool,
            tc.tile_pool(name="p2_pool", bufs=2, space="PSUM") as p2_pool,
        ):
            w1sb = singles.tile([128, NW, 4, CH], F32)
            nc.sync.dma_start(out=w1sb, in_=w1rep[:, :, :, :])
            w2sb = singles.tile([128, NW, 32], F32)
            nc.sync.dma_start(out=w2sb, in_=w2blk[:, :, :])
            b1sb = singles.tile([128, NW], F32)
            nc.sync.dma_start(out=b1sb, in_=b1col[:, :])
            b2sb = singles.tile([128, 1], F32)
            nc.sync.dma_start(out=b2sb, in_=b2col[:, :])

            for cc in range(NCHUNK):
                c0 = cc * C
                # ---- load x in halves, relu, transpose into xt ----
                xt = xtp.tile([128, NSUB, NODES, CH], F32)
                for hh in range(2):
                    xh = xp.tile([128, HSUB, NODES, CH], F32)
                    lo = c0 + hh * (C // 2)
                    nc.sync.dma_start(
                        out=xh.rearrange("p s n c -> p s (n c)"),
                        in_=h[lo:lo + C // 2, :].rearrange(
                            "(s p) f -> p s f", p=128),
                    )
                    qq = HSUB // 2
                    nc.gpsimd.tensor_scalar_max(
                        xh[:, 0:qq].rearrange("p s n c -> p (s n c)"),
                        xh[:, 0:qq].rearrange("p s n c -> p (s n c)"), 0.0)
                    nc.scalar.activation(
                        xh[:, qq:HSUB].rearrange("p s n c -> p (s n c)"),
                        xh[:, qq:HSUB].rearrange("p s n c -> p (s n c)"),
                        mybir.ActivationFunctionType.Relu)
                    nc.vector.transpose(
                        xt[:, hh * HSUB:(hh + 1) * HSUB], xh)

                # ---- valid -> group mask (natural layout) ----
                vi = vp.tile([128, NSUB, NODES], I32)
                nc.scalar.dma_start(
                    out=vi,
                    in_=valid[c0:c0 + C, :].rearrange("(s p) n -> p s n", p=128),
                )
                vf = vp.tile([128, NSUB, NODES], F32)
                nc.gpsimd.tensor_copy(vf, vi)
                gv = vp.tile([128, NSUB, 8], F32)
                nc.vector.tensor_reduce(
                    gv[:, :, 0:3],
                    vf[:, :, 0:12].rearrange("p s (k g) -> p s g k", g=3),
                    axis=mybir.AxisListType.X, op=mybir.AluOpType.add)
                nc.vector.tensor_reduce(
                    gv[:, :, 3:4], vf[:, :, 12:16],
                    axis=mybir.AxisListType.X, op=mybir.AluOpType.add)
                nc.vector.tensor_reduce(
                    gv[:, :, 4:6],
                    vf[:, :, 16:24].rearrange("p s (k g) -> p s g k", g=2),
                    axis=mybir.AxisListType.X, op=mybir.AluOpType.add)
                nc.gpsimd.tensor_scalar(
                    gv[:, :, 0:6], gv[:, :, 0:6], 0.5, None,
                    op0=mybir.AluOpType.is_gt)
                mask_nat = vp.tile([128, NSUB, 32], F32)
                nc.gpsimd.memset(mask_nat[:, :, 24:32], 0.0)
                nc.gpsimd.tensor_copy(
                    mask_nat[:, :, 0:12].rearrange("p s (k g) -> p s g k", g=3),
                    gv[:, :, 0:3].broadcast_to([128, NSUB, 3, 4]))
                nc.gpsimd.tensor_copy(
                    mask_nat[:, :, 12:16],
                    gv[:, :, 3:4].broadcast_to([128, NSUB, 4]))
                nc.gpsimd.tensor_copy(
                    mask_nat[:, :, 16:24].rearrange("p s (k g) -> p s g k", g=2),
                    gv[:, :, 4:6].broadcast_to([128, NSUB, 2, 4]))
                maskT = vp.tile([128, NSUB, 32], F32)
                nc.vector.transpose(maskT, mask_nat)

                # ---- the 2-layer MLP on TensorE ----
                p2 = p2_pool.tile([128, NSUB, 32], F32)  # [(i,n), (sub,si)]
                p2f = p2.rearrange("p a b -> p (a b)")
                for w in range(NW):
                    pha = pha_pool.tile([128, 2 * CQ], F32)  # i = 0, 1
                    phb = phb_pool.tile([128, 2 * CQ], F32)  # i = 2, 3
                    for i in range(4):
                        ph_t = pha if i < 2 else phb
                        fo = (i % 2) * CQ
                        for jj in range(4):
                            n = 4 * w + jj
                            nc.tensor.matmul(
                                ph_t[32 * jj:32 * jj + 32, fo:fo + CQ],
                                lhsT=w1sb[32 * i:32 * i + 32, w, jj, :],
                                rhs=xt[32 * i:32 * i + 32, :, n, :],
                                start=True, stop=True,
                                tile_position=(32 * i, 32 * jj))
                    hid = hidp.tile([128, C], F32)
                    nc.scalar.activation(
                        hid[:, 0:2 * CQ], pha,
                        mybir.ActivationFunctionType.Relu,
                        bias=b1sb[:, w:w + 1])
                    nc.scalar.activation(
                        hid[:, 2 * CQ:4 * CQ], phb,
                        mybir.ActivationFunctionType.Relu,
                        bias=b1sb[:, w:w + 1])
                    for i in range(4):
                        nc.tensor.matmul(
                            p2f[32 * i:32 * i + 32, :],
                            lhsT=w2sb[:, w, :],
                            rhs=hid[:, i * CQ:(i + 1) * CQ],
                            start=(w == 0), stop=(w == NW - 1),
                            skip_group_check=True,
                            tile_position=(0, 32 * i))

                # ---- +b2, mask, transpose back, store ----
                m2 = op.tile([128, NSUB, 32], F32)
                nc.vector.tensor_scalar(
                    m2, p2, b2sb[:, 0:1], None, op0=mybir.AluOpType.add)
                nc.vector.tensor_tensor(
                    m2, m2, maskT, op=mybir.AluOpType.mult)
                outT = op.tile([128, NSUB, 32], F32)
                nc.vector.transpose(outT, m2)
                nc.scalar.dma_start(
                    out=out[c0:c0 + C, :].rearrange("(s p) n -> p s n", p=128),
                    in_=outT[:, :, 0:NODES],
                )

    nc.compile()
    return nc


_PROGRAM = None


def _get_program():
    global _PROGRAM
    if _PROGRAM is None:
        _PROGRAM = _build_program()
    return _PROGRAM


def _prep_weights(W1, b1, W2, b2):
    flat = GROUPING.reshape(-1)
    g_of = np.zeros(NODES, np.int64)
    k_of = np.zeros(NODES, np.int64)
    for q, nid in enumerate(flat):
        g_of[nid] = q // 4
        k_of[nid] = q % 4
    W1n = np.ascontiguousarray(W1[g_of, k_of]).astype(np.float32)  # [24,32,32]
    W2n = np.ascontiguousarray(W2[g_of, k_of]).astype(np.float32)  # [24,32,1]
    b1n = np.ascontiguousarray(b1[g_of, k_of]).astype(np.float32)  # [24,32]
    b2n = np.ascontiguousarray(b2[g_of, k_of]).astype(np.float32)  # [24,1]

    w1rep = np.zeros((128, NW, 4, CH), np.float32)
    w2blk = np.zeros((128, NW, 32), np.float32)
    b1col = np.zeros((128, NW), np.float32)
    b2col = np.zeros((128, 1), np.float32)
    for w in range(NW):
        for jj in range(4):
            n = 4 * w + jj
            for r in range(4):
                w1rep[32 * r:32 * r + 32, w, jj, :] = W1n[n]
            w2blk[32 * jj:32 * jj + 32, w, n] = W2n[n][:, 0]
            b1col[32 * jj:32 * jj + 32, w] = b1n[n]
    for i in range(4):
        b2col[32 * i:32 * i + 24, 0] = b2n[:, 0]
    return w1rep, w2blk, b1col, b2col


def _make_in_maps(inputs):
    w1rep, w2blk, b1col, b2col = _prep_weights(
        inputs["W1"], inputs["b1"], inputs["W2"], inputs["b2"])
    h2 = np.ascontiguousarray(
        inputs["h"], dtype=np.float32).reshape(S_TOT, NODES * CH)
    v2 = np.ascontiguousarray(
        inputs["valid"], dtype=np.int32).reshape(S_TOT, NODES)

    in_maps = []
    for c in range(N_CORES):
        sl = slice(c * S, (c + 1) * S)
        in_maps.append({
            "h": h2[sl],
            "valid": v2[sl],
            "w1rep": w1rep,
            "w2blk": w2blk,
            "b1col": b1col,
            "b2col": b2col,
        })
    return in_maps


def kernel(h, valid, W1, b1, W2, b2):
    nc = _get_program()
    in_maps = _make_in_maps(dict(h=h, valid=valid, W1=W1, b1=b1, W2=W2, b2=b2))
    res = run_bass_kernel_spmd(nc, in_maps, core_ids=list(range(N_CORES)))
    outs = [res.results[c]["out"] for c in range(N_CORES)]
    full = np.concatenate(outs, axis=0).astype(np.float32)
    return full.reshape(S_TOT, NODES, 1)



# revision 4
# speedup vs baseline: 3.7326x; 3.7326x over previous
"""GroupedPNMLP forward on 8 Trainium2 NeuronCores (pure data parallel).

Per-node 2-layer MLP (32->32->1), 24 nodes in 6 groups of 4, with a
group-validity mask and node permutation.  Full inputs in, full output out;
samples are sharded N/8 per core, tiny weights replicated.

Per-core pipeline (chunks of C=2048 samples):
  DMA h in two half-chunks (natural [s, n*c] layout, 3072B descriptors)
  -> ReLU + fp32->bf16 convert, split DVE (2x_2P mode) / ACT
  -> DVE 32x32 block-transpose (bf16, channels onto partitions)
  -> mm1: 16 concurrent 32x32 bf16 TensorE sub-tile matmuls (tile_position);
     per-node weights replicated across the 4 row blocks; 6 waves of 4 nodes
  -> ACT: fused ReLU+b1 PSUM->SBUF copy (bf16 out)
  -> mm2: block-structured W2 [128x32] bf16, accumulated over the 6 waves
     into one PSUM bank [128, 512]; b2 added via a rank-1 (K=1) matmul
  -> DVE transpose PSUM->SBUF back to sample-major, x group-valid mask
  -> DMA out packed [128, NSUB, 32] (2048B descriptors; host unpacks)

valid is host-repacked to partition-major [128, NCHUNK, NSUB, 24] so its
DMA uses 1536B descriptors; group-validity mask computed on DVE/GpSimd.
"""

import numpy as np
import ml_dtypes

import concourse.bass as bass
from concourse import bacc
import concourse.tile as tile
from concourse import mybir
from concourse.bass_utils import run_bass_kernel_spmd

F32 = mybir.dt.float32
BF16 = mybir.dt.bfloat16
I32 = mybir.dt.int32

GROUPING = np.array(
    [[0, 3, 6, 9], [1, 4, 7, 10], [2, 5, 8, 11],
     [12, 13, 14, 15], [16, 18, 20, 22], [17, 19, 21, 23]], dtype=np.int32)

N_CORES = 8
S_TOT = 131072
S = S_TOT // N_CORES      # 16384 samples per core
NODES = 24
CH = 32                   # in channels = hidden dim
C = 2048                  # samples per chunk
NSUB = C // 128           # 16 sub-tiles of 128 samples
HSUB = NSUB // 2          # 8 sub-tiles per half-chunk
CQ = C // 4               # 512 samples per i-block = one matmul N = one bank
NCHUNK = S // C           # 8
NW = 6                    # waves of 4 nodes
HFD = HSUB * NODES * CH   # 6144 flattened free elems per half-chunk
RSPLIT = 3392             # leading elems of each half-chunk relu'd on DVE


def _build_program():
    nc = bacc.Bacc(None, target_bir_lowering=False)

    h = nc.dram_tensor("h", [S, NODES * CH], F32, kind="ExternalInput")
    vpk = nc.dram_tensor("vpk", [128, NCHUNK, NSUB, NODES], I32,
                         kind="ExternalInput")
    w1rep = nc.dram_tensor("w1rep", [128, NW, 4, CH], BF16,
                           kind="ExternalInput")
    w2blk = nc.dram_tensor("w2blk", [128, NW, 32], BF16, kind="ExternalInput")
    b1col = nc.dram_tensor("b1col", [128, NW], F32, kind="ExternalInput")
    b2row = nc.dram_tensor("b2row", [1, 128], BF16, kind="ExternalInput")
    out = nc.dram_tensor("out", [128, NCHUNK, NSUB, 32], F32,
                         kind="ExternalOutput")

    with tile.TileContext(nc) as tc:
        with (
            tc.tile_pool(name="singles", bufs=1) as singles,
            tc.tile_pool(name="xp", bufs=2) as xp,
            tc.tile_pool(name="xbp", bufs=2) as xbp,
            tc.tile_pool(name="xtp", bufs=2) as xtp,
            tc.tile_pool(name="hidp", bufs=2) as hidp,
            tc.tile_pool(name="vp", bufs=2) as vp,
            tc.tile_pool(name="op", bufs=2) as op,
            tc.tile_pool(name="pha_pool", bufs=2, space="PSUM") as pha_pool,
            tc.tile_pool(name="phb_pool", bufs=1, space="PSUM") as phb_pool,
            tc.tile_pool(name="p2_pool", bufs=2, space="PSUM") as p2_pool,
        ):
            w1sb = singles.tile([128, NW, 4, CH], BF16)
            nc.sync.dma_start(out=w1sb, in_=w1rep[:, :, :, :])
            w2sb = singles.tile([128, NW, 32], BF16)
            nc.sync.dma_start(out=w2sb, in_=w2blk[:, :, :])
            b1sb = singles.tile([128, NW], F32)
            nc.sync.dma_start(out=b1sb, in_=b1col[:, :])
            b2sb = singles.tile([1, 128], BF16)
            nc.sync.dma_start(out=b2sb, in_=b2row[:, :])
            ones = singles.tile([1, CQ], BF16)
            nc.vector.memset(ones, 1.0)

            for cc in range(NCHUNK):
                c0 = cc * C
                # ---- load x in halves; relu+bf16 split DVE/ACT; transpose ----
                xt = xtp.tile([128, NSUB, NODES, CH], BF16)
                for hh in range(2):
                    xh = xp.tile([128, HSUB, NODES, CH], F32)
                    lo = c0 + hh * (C // 2)
                    nc.sync.dma_start(
                        out=xh.rearrange("p s n c -> p s (n c)"),
                        in_=h[lo:lo + C // 2, :].rearrange(
                            "(s p) f -> p s f", p=128),
                    )
                    xhb = xbp.tile([128, HSUB, NODES, CH], BF16)
                    xhf = xh.rearrange("p s n c -> p (s n c)")
                    xbf = xhb.rearrange("p s n c -> p (s n c)")
                    nc.vector.tensor_scalar(
                        xbf[:, 0:RSPLIT], xhf[:, 0:RSPLIT], 0.0, None,
                        op0=mybir.AluOpType.max)
                    nc.scalar.activation(
                        xbf[:, RSPLIT:HFD], xhf[:, RSPLIT:HFD],
                        mybir.ActivationFunctionType.Relu)
                    nc.vector.transpose(
                        xt[:, hh * HSUB:(hh + 1) * HSUB], xhb)

                # ---- valid (packed) -> group mask in natural layout ----
                vi = vp.tile([128, NSUB, NODES], I32)
                nc.scalar.dma_start(out=vi, in_=vpk[:, cc])
                gv = vp.tile([128, NSUB, 8], F32)
                nc.vector.tensor_reduce(
                    gv[:, :, 0:3],
                    vi[:, :, 0:12].rearrange("p s (k g) -> p s g k", g=3),
                    axis=mybir.AxisListType.X, op=mybir.AluOpType.add)
                nc.vector.tensor_reduce(
                    gv[:, :, 3:4], vi[:, :, 12:16],
                    axis=mybir.AxisListType.X, op=mybir.AluOpType.add)
                nc.vector.tensor_reduce(
                    gv[:, :, 4:6],
                    vi[:, :, 16:24].rearrange("p s (k g) -> p s g k", g=2),
                    axis=mybir.AxisListType.X, op=mybir.AluOpType.add)
                nc.vector.tensor_scalar(
                    gv[:, :, 0:6], gv[:, :, 0:6], 0.5, None,
                    op0=mybir.AluOpType.is_gt)
                mask_nat = vp.tile([128, NSUB, NODES], F32)
                nc.gpsimd.tensor_copy(
                    mask_nat[:, :, 0:12].rearrange("p s (k g) -> p s g k", g=3),
                    gv[:, :, 0:3].broadcast_to([128, NSUB, 3, 4]))
                nc.gpsimd.tensor_copy(
                    mask_nat[:, :, 12:16],
                    gv[:, :, 3:4].broadcast_to([128, NSUB, 4]))
                nc.gpsimd.tensor_copy(
                    mask_nat[:, :, 16:24].rearrange("p s (k g) -> p s g k", g=2),
                    gv[:, :, 4:6].broadcast_to([128, NSUB, 2, 4]))

                # ---- the 2-layer MLP on TensorE ----
                p2 = p2_pool.tile([128, NSUB, 32], F32)  # [(i,n), (sub,si)]
                p2f = p2.rearrange("p a b -> p (a b)")
                for w in range(NW):
                    pha = pha_pool.tile([128, 2 * CQ], F32)  # i = 0, 1
                    phb = phb_pool.tile([128, 2 * CQ], F32)  # i = 2, 3
                    for i in range(4):
                        ph_t = pha if i < 2 else phb
                        fo = (i % 2) * CQ
                        for jj in range(4):
                            n = 4 * w + jj
                            nc.tensor.matmul(
                                ph_t[32 * jj:32 * jj + 32, fo:fo + CQ],
                                lhsT=w1sb[32 * i:32 * i + 32, w, jj, :],
                                rhs=xt[32 * i:32 * i + 32, :, n, :],
                                start=True, stop=True,
                                tile_position=(32 * i, 32 * jj))
                    hid = hidp.tile([128, C], BF16)
                    nc.scalar.activation(
                        hid[:, 0:2 * CQ], pha,
                        mybir.ActivationFunctionType.Relu,
                        bias=b1sb[:, w:w + 1])
                    nc.scalar.activation(
                        hid[:, 2 * CQ:4 * CQ], phb,
                        mybir.ActivationFunctionType.Relu,
                        bias=b1sb[:, w:w + 1])
                    for i in range(4):
                        nc.tensor.matmul(
                            p2f[32 * i:32 * i + 32, :],
                            lhsT=w2sb[:, w, :],
                            rhs=hid[:, i * CQ:(i + 1) * CQ],
                            start=(w == 0), stop=False,
                            skip_group_check=True,
                            tile_position=(0, 32 * i))
                # += b2 (rank-1 matmul: b2 per out-partition x ones row)
                nc.tensor.matmul(
                    p2f,
                    lhsT=b2sb[:, :],
                    rhs=ones[:, :],
                    start=False, stop=True,
                    skip_group_check=True,
                    tile_position=(0, 0))

                # ---- transpose back (PSUM src), mask, store packed ----
                outT = op.tile([128, NSUB, 32], F32)
                nc.vector.transpose(outT, p2)
                nc.vector.tensor_tensor(
                    outT[:, :, 0:NODES], outT[:, :, 0:NODES], mask_nat,
                    op=mybir.AluOpType.mult)
                nc.scalar.dma_start(out=out[:, cc], in_=outT)

    nc.compile()
    return nc


_PROGRAM = None


def _get_program():
    global _PROGRAM
    if _PROGRAM is None:
        _PROGRAM = _build_program()
    return _PROGRAM


def _prep_weights(W1, b1, W2, b2):
    flat = GROUPING.reshape(-1)
    g_of = np.zeros(NODES, np.int64)
    k_of = np.zeros(NODES, np.int64)
    for q, nid in enumerate(flat):
        g_of[nid] = q // 4
        k_of[nid] = q % 4
    W1n = np.ascontiguousarray(W1[g_of, k_of]).astype(np.float32)  # [24,32,32]
    W2n = np.ascontiguousarray(W2[g_of, k_of]).astype(np.float32)  # [24,32,1]
    b1n = np.ascontiguousarray(b1[g_of, k_of]).astype(np.float32)  # [24,32]
    b2n = np.ascontiguousarray(b2[g_of, k_of]).astype(np.float32)  # [24,1]

    w1rep = np.zeros((128, NW, 4, CH), np.float32)
    w2blk = np.zeros((128, NW, 32), np.float32)
    b1col = np.zeros((128, NW), np.float32)
    b2row = np.zeros((1, 128), np.float32)
    for w in range(NW):
        for jj in range(4):
            n = 4 * w + jj
            for r in range(4):
                w1rep[32 * r:32 * r + 32, w, jj, :] = W1n[n]
            w2blk[32 * jj:32 * jj + 32, w, n] = W2n[n][:, 0]
            b1col[32 * jj:32 * jj + 32, w] = b1n[n]
    for i in range(4):
        b2row[0, 32 * i:32 * i + 24] = b2n[:, 0]
    return (w1rep.astype(ml_dtypes.bfloat16),
            w2blk.astype(ml_dtypes.bfloat16),
            b1col,
            b2row.astype(ml_dtypes.bfloat16))


def _make_in_maps(inputs):
    w1rep, w2blk, b1col, b2row = _prep_weights(
        inputs["W1"], inputs["b1"], inputs["W2"], inputs["b2"])
    h2 = np.ascontiguousarray(
        inputs["h"], dtype=np.float32).reshape(S_TOT, NODES * CH)
    v2 = np.ascontiguousarray(
        inputs["valid"], dtype=np.int32).reshape(S_TOT, NODES)

    in_maps = []
    for c in range(N_CORES):
        sl = slice(c * S, (c + 1) * S)
        # pack valid partition-major: vpk[p, cc, ss, n] = v[cc*C + 128*ss + p, n]
        vpk = np.ascontiguousarray(
            v2[sl].reshape(NCHUNK, NSUB, 128, NODES).transpose(2, 0, 1, 3))
        in_maps.append({
            "h": h2[sl],
            "vpk": vpk,
            "w1rep": w1rep,
            "w2blk": w2blk,
            "b1col": b1col,
            "b2row": b2row,
        })
    return in_maps


def kernel(h, valid, W1, b1, W2, b2):
    nc = _get_program()
    in_maps = _make_in_maps(dict(h=h, valid=valid, W1=W1, b1=b1, W2=W2, b2=b2))
    res = run_bass_kernel_spmd(nc, in_maps, core_ids=list(range(N_CORES)))
    outs = []
    for c in range(N_CORES):
        arr = res.results[c]["out"]  # [128, NCHUNK, NSUB, 32]
        outs.append(np.ascontiguousarray(
            arr.transpose(1, 2, 0, 3)).reshape(S, 32)[:, :NODES])
    full = np.concatenate(outs, axis=0).astype(np.float32)
    return full.reshape(S_TOT, NODES, 1)


# revision 5
# speedup vs baseline: 3.7375x; 1.0013x over previous
"""GroupedPNMLP forward on 8 Trainium2 NeuronCores (pure data parallel).

Per-node 2-layer MLP (32->32->1), 24 nodes in 6 groups of 4, with a
group-validity mask and node permutation.  Full inputs in, full output out;
samples are sharded N/8 per core, tiny weights replicated.

Per-core pipeline (chunks of C=2048 samples):
  DMA h in two half-chunks (natural [s, n*c] layout, 3072B descriptors)
  -> ReLU + fp32->bf16 convert, split DVE (2x_2P mode) / ACT
  -> DVE 32x32 block-transpose (bf16, channels onto partitions)
  -> mm1: 16 concurrent 32x32 bf16 TensorE sub-tile matmuls (tile_position);
     per-node weights replicated across the 4 row blocks; 6 waves of 4 nodes
  -> ACT: fused ReLU+b1 PSUM->SBUF copy (bf16 out)
  -> mm2: block-structured W2 [128x32] bf16, accumulated over the 6 waves
     into one PSUM bank [128, 512]; b2 added via a rank-1 (K=1) matmul
  -> DVE transpose PSUM->SBUF back to sample-major, x group-valid mask
  -> DMA out packed [128, NSUB, 32] (2048B descriptors; host unpacks)

valid is host-repacked to partition-major [128, NCHUNK, NSUB, 24] so its
DMA uses 1536B descriptors; group-validity mask computed on DVE/GpSimd.
"""

import numpy as np
import ml_dtypes

import concourse.bass as bass
from concourse import bacc
import concourse.tile as tile
from concourse import mybir
from concourse.bass_utils import run_bass_kernel_spmd

F32 = mybir.dt.float32
BF16 = mybir.dt.bfloat16
I32 = mybir.dt.int32

GROUPING = np.array(
    [[0, 3, 6, 9], [1, 4, 7, 10], [2, 5, 8, 11],
     [12, 13, 14, 15], [16, 18, 20, 22], [17, 19, 21, 23]], dtype=np.int32)

N_CORES = 8
S_TOT = 131072
S = S_TOT // N_CORES      # 16384 samples per core
NODES = 24
CH = 32                   # in channels = hidden dim
C = 2048                  # samples per chunk
NSUB = C // 128           # 16 sub-tiles of 128 samples
HSUB = NSUB // 2          # 8 sub-tiles per half-chunk
CQ = C // 4               # 512 samples per i-block = one matmul N = one bank
NCHUNK = S // C           # 8
NW = 6                    # waves of 4 nodes
HFD = HSUB * NODES * CH   # 6144 flattened free elems per half-chunk
RSPLIT = 3392             # leading elems of each half-chunk relu'd on DVE


def _build_program():
    nc = bacc.Bacc(None, target_bir_lowering=False)

    h = nc.dram_tensor("h", [S, NODES * CH], F32, kind="ExternalInput")
    vpk = nc.dram_tensor("vpk", [128, NCHUNK, NSUB, NODES], I32,
                         kind="ExternalInput")
    w1rep = nc.dram_tensor("w1rep", [128, NW, 4, CH], BF16,
                           kind="ExternalInput")
    w2blk = nc.dram_tensor("w2blk", [128, NW, 32], BF16, kind="ExternalInput")
    b1col = nc.dram_tensor("b1col", [128, NW], F32, kind="ExternalInput")
    b2row = nc.dram_tensor("b2row", [1, 128], BF16, kind="ExternalInput")
    out = nc.dram_tensor("out", [128, NCHUNK, NSUB, 32], F32,
                         kind="ExternalOutput")

    with tile.TileContext(nc) as tc:
        with (
            tc.tile_pool(name="singles", bufs=1) as singles,
            tc.tile_pool(name="xp", bufs=2) as xp,
            tc.tile_pool(name="xbp", bufs=2) as xbp,
            tc.tile_pool(name="xtp", bufs=2) as xtp,
            tc.tile_pool(name="hidp", bufs=2) as hidp,
            tc.tile_pool(name="vp", bufs=2) as vp,
            tc.tile_pool(name="op", bufs=2) as op,
            tc.tile_pool(name="pha_pool", bufs=2, space="PSUM") as pha_pool,
            tc.tile_pool(name="phb_pool", bufs=1, space="PSUM") as phb_pool,
            tc.tile_pool(name="p2_pool", bufs=2, space="PSUM") as p2_pool,
        ):
            w1sb = singles.tile([128, NW, 4, CH], BF16)
            nc.sync.dma_start(out=w1sb, in_=w1rep[:, :, :, :])
            w2sb = singles.tile([128, NW, 32], BF16)
            nc.sync.dma_start(out=w2sb, in_=w2blk[:, :, :])
            b1sb = singles.tile([128, NW], F32)
            nc.sync.dma_start(out=b1sb, in_=b1col[:, :])
            b2sb = singles.tile([1, 128], BF16)
            nc.sync.dma_start(out=b2sb, in_=b2row[:, :])
            ones = singles.tile([1, CQ], BF16)
            nc.vector.memset(ones, 1.0)

            def emit_loads(cc):
                c0 = cc * C
                xhs = []
                for hh in range(2):
                    xh = xp.tile([128, HSUB, NODES, CH], F32)
                    lo = c0 + hh * (C // 2)
                    nc.sync.dma_start(
                        out=xh.rearrange("p s n c -> p s (n c)"),
                        in_=h[lo:lo + C // 2, :].rearrange(
                            "(s p) f -> p s f", p=128),
                    )
                    xhs.append(xh)
                vi = vp.tile([128, NSUB, NODES], I32)
                nc.scalar.dma_start(out=vi, in_=vpk[:, cc])
                return {"xhs": xhs, "vi": vi}

            def emit_input(st):
                # relu + fp32->bf16 (DVE 2x / ACT split), 32x32 transpose
                xt = xtp.tile([128, NSUB, NODES, CH], BF16)
                for hh in range(2):
                    xh = st["xhs"][hh]
                    xhb = xbp.tile([128, HSUB, NODES, CH], BF16)
                    xhf = xh.rearrange("p s n c -> p (s n c)")
                    xbf = xhb.rearrange("p s n c -> p (s n c)")
                    nc.vector.tensor_scalar(
                        xbf[:, 0:RSPLIT], xhf[:, 0:RSPLIT], 0.0, None,
                        op0=mybir.AluOpType.max)
                    nc.scalar.activation(
                        xbf[:, RSPLIT:HFD], xhf[:, RSPLIT:HFD],
                        mybir.ActivationFunctionType.Relu)
                    nc.vector.transpose(
                        xt[:, hh * HSUB:(hh + 1) * HSUB], xhb)
                st["xt"] = xt

                # group mask: max over each group's nodes (valid is 0/1)
                vi = st["vi"]
                gv = vp.tile([128, NSUB, 8], F32)
                nc.vector.tensor_reduce(
                    gv[:, :, 0:3],
                    vi[:, :, 0:12].rearrange("p s (k g) -> p s g k", g=3),
                    axis=mybir.AxisListType.X, op=mybir.AluOpType.max)
                nc.vector.tensor_reduce(
                    gv[:, :, 3:4], vi[:, :, 12:16],
                    axis=mybir.AxisListType.X, op=mybir.AluOpType.max)
                nc.vector.tensor_reduce(
                    gv[:, :, 4:6],
                    vi[:, :, 16:24].rearrange("p s (k g) -> p s g k", g=2),
                    axis=mybir.AxisListType.X, op=mybir.AluOpType.max)
                mask_nat = vp.tile([128, NSUB, NODES], F32)
                nc.gpsimd.tensor_copy(
                    mask_nat[:, :, 0:12].rearrange("p s (k g) -> p s g k", g=3),
                    gv[:, :, 0:3].broadcast_to([128, NSUB, 3, 4]))
                nc.gpsimd.tensor_copy(
                    mask_nat[:, :, 12:16],
                    gv[:, :, 3:4].broadcast_to([128, NSUB, 4]))
                nc.gpsimd.tensor_copy(
                    mask_nat[:, :, 16:24].rearrange("p s (k g) -> p s g k", g=2),
                    gv[:, :, 4:6].broadcast_to([128, NSUB, 2, 4]))
                st["mask_nat"] = mask_nat

            def emit_mm(st, cc):
                xt = st["xt"]
                p2 = p2_pool.tile([128, NSUB, 32], F32)  # [(i,n), (sub,si)]
                p2f = p2.rearrange("p a b -> p (a b)")
                for w in range(NW):
                    pha = pha_pool.tile([128, 2 * CQ], F32)  # i = 0, 1
                    phb = phb_pool.tile([128, 2 * CQ], F32)  # i = 2, 3
                    for i in range(4):
                        ph_t = pha if i < 2 else phb
                        fo = (i % 2) * CQ
                        for jj in range(4):
                            n = 4 * w + jj
                            nc.tensor.matmul(
                                ph_t[32 * jj:32 * jj + 32, fo:fo + CQ],
                                lhsT=w1sb[32 * i:32 * i + 32, w, jj, :],
                                rhs=xt[32 * i:32 * i + 32, :, n, :],
                                start=True, stop=True,
                                tile_position=(32 * i, 32 * jj))
                    hid = hidp.tile([128, C], BF16)
                    nc.scalar.activation(
                        hid[:, 0:2 * CQ], pha,
                        mybir.ActivationFunctionType.Relu,
                        bias=b1sb[:, w:w + 1])
                    nc.scalar.activation(
                        hid[:, 2 * CQ:4 * CQ], phb,
                        mybir.ActivationFunctionType.Relu,
                        bias=b1sb[:, w:w + 1])
                    for i in range(4):
                        nc.tensor.matmul(
                            p2f[32 * i:32 * i + 32, :],
                            lhsT=w2sb[:, w, :],
                            rhs=hid[:, i * CQ:(i + 1) * CQ],
                            start=(w == 0), stop=False,
                            skip_group_check=True,
                            tile_position=(0, 32 * i))
                # += b2 (rank-1 matmul: b2 per out-partition x ones row)
                nc.tensor.matmul(
                    p2f,
                    lhsT=b2sb[:, :],
                    rhs=ones[:, :],
                    start=False, stop=True,
                    skip_group_check=True,
                    tile_position=(0, 0))

                # transpose back (PSUM src), mask, store packed
                outT = op.tile([128, NSUB, 32], F32)
                nc.vector.transpose(outT, p2)
                nc.vector.tensor_tensor(
                    outT[:, :, 0:NODES], outT[:, :, 0:NODES], st["mask_nat"],
                    op=mybir.AluOpType.mult)
                nc.scalar.dma_start(out=out[:, cc], in_=outT)

            # 3-stage software pipeline: loads(k) | input(k-1) | mm+tail(k-2)
            st = [None] * NCHUNK
            for it in range(NCHUNK + 2):
                if it < NCHUNK:
                    st[it] = emit_loads(it)
                if 1 <= it <= NCHUNK:
                    emit_input(st[it - 1])
                if 2 <= it <= NCHUNK + 1:
                    emit_mm(st[it - 2], it - 2)
                    st[it - 2] = None

    nc.compile()
    return nc


_PROGRAM = None


def _get_program():
    global _PROGRAM
    if _PROGRAM is None:
        _PROGRAM = _build_program()
    return _PROGRAM


def _prep_weights(W1, b1, W2, b2):
    flat = GROUPING.reshape(-1)
    g_of = np.zeros(NODES, np.int64)
    k_of = np.zeros(NODES, np.int64)
    for q, nid in enumerate(flat):
        g_of[nid] = q // 4
        k_of[nid] = q % 4
    W1n = np.ascontiguousarray(W1[g_of, k_of]).astype(np.float32)  # [24,32,32]
    W2n = np.ascontiguousarray(W2[g_of, k_of]).astype(np.float32)  # [24,32,1]
    b1n = np.ascontiguousarray(b1[g_of, k_of]).astype(np.float32)  # [24,32]
    b2n = np.ascontiguousarray(b2[g_of, k_of]).astype(np.float32)  # [24,1]

    w1rep = np.zeros((128, NW, 4, CH), np.float32)
    w2blk = np.zeros((128, NW, 32), np.float32)
    b1col = np.zeros((128, NW), np.float32)
    b2row = np.zeros((1, 128), np.float32)
    for w in range(NW):
        for jj in range(4):
            n = 4 * w + jj
            for r in range(4):
                w1rep[32 * r:32 * r + 32, w, jj, :] = W1n[n]
            w2blk[32 * jj:32 * jj + 32, w, n] = W2n[n][:, 0]
            b1col[32 * jj:32 * jj + 32, w] = b1n[n]
    for i in range(4):
        b2row[0, 32 * i:32 * i + 24] = b2n[:, 0]
    return (w1rep.astype(ml_dtypes.bfloat16),
            w2blk.astype(ml_dtypes.bfloat16),
            b1col,
            b2row.astype(ml_dtypes.bfloat16))


def _make_in_maps(inputs):
    w1rep, w2blk, b1col, b2row = _prep_weights(
        inputs["W1"], inputs["b1"], inputs["W2"], inputs["b2"])
    h2 = np.ascontiguousarray(
        inputs["h"], dtype=np.float32).reshape(S_TOT, NODES * CH)
    v2 = np.ascontiguousarray(
        inputs["valid"], dtype=np.int32).reshape(S_TOT, NODES)

    in_maps = []
    for c in range(N_CORES):
        sl = slice(c * S, (c + 1) * S)
        # pack valid partition-major: vpk[p, cc, ss, n] = v[cc*C + 128*ss + p, n]
        vpk = np.ascontiguousarray(
            v2[sl].reshape(NCHUNK, NSUB, 128, NODES).transpose(2, 0, 1, 3))
        in_maps.append({
            "h": h2[sl],
            "vpk": vpk,
            "w1rep": w1rep,
            "w2blk": w2blk,
            "b1col": b1col,
            "b2row": b2row,
        })
    return in_maps


def kernel(h, valid, W1, b1, W2, b2):
    nc = _get_program()
    in_maps = _make_in_maps(dict(h=h, valid=valid, W1=W1, b1=b1, W2=W2, b2=b2))
    res = run_bass_kernel_spmd(nc, in_maps, core_ids=list(range(N_CORES)))
    outs = []
    for c in range(N_CORES):
        arr = res.results[c]["out"]  # [128, NCHUNK, NSUB, 32]
        outs.append(np.ascontiguousarray(
            arr.transpose(1, 2, 0, 3)).reshape(S, 32)[:, :NODES])
    full = np.concatenate(outs, axis=0).astype(np.float32)
    return full.reshape(S_TOT, NODES, 1)
